# revision 3
# baseline (speedup 1.0000x reference)
"""Trainium2 Bass kernel for spherical deep GMM classifier (DGMMC).

Reference (B=8192, D=1024, C=128 classes, K=8 comps, N=C*K=1024):
    bw = clip(bandwidths, 1e-3, 100); a = 1/bw
    log_prob[b,n] = -0.5*(D*log(2pi) + D*log(bw[n]) + sq_dist[b,n]/bw[n])
    log_prob += log_softmax(weights.reshape(C,K),1).reshape(N)
    lse1[b,c]  = LSE_k(log_prob[b,c*K+k]) + log_softmax(priors)[c]
    out[b,c]   = lse1[b,c] - LSE_c(lse1[b,c])

Data-parallel over batch across 8 cores; per-component affine terms folded
into an augmented GEMM assembled on the host (see _host_prep).

Performance structure:
  - Hybrid-precision GEMM: n8 chunks of 256 contraction rows run as
    fp8e4m3 DoubleRow matmuls (2 contraction rows/PE cell -> half the
    passes of fp16), remaining rows + the rank-2 augmentation stay fp16.
    Hybrid quantization error ~ 2.07e-2 * sqrt(fp8 fraction) ~ 1.47e-2.
  - 12 warm-up matmuls bridge the DMA head so the PE HAM clock-gate
    reaches 8/8 before the real stream (DMA-paced early matmuls alone
    leave it stuck at 4/8).
  - Epilogue split into a per-tile "front" (grouped max / shift / exp /
    gpsimd pairwise-sum tree) and a "row-LSE" emitted two tiles later so
    every op's inputs are ready when it reaches its engine's FIFO head.
  - Row-LSE uses s2 = sum_c gsum_c * exp(gmax_c - rmax) (one DVE
    scalar_tensor_tensor with accumulate) so no op chain runs through
    lse1 before the final affine.
  - Last b-tile runs as two staggered 512-wide half groups so its first
    half's epilogue overlaps the second half's matmuls.
  - fp16 output DMA (upcast on host; quantization adds <1e-3 rel err).
"""

import math

import numpy as np
import ml_dtypes

B, D, C, K = 8192, 1024, 128, 8
N = C * K
NCORES = 8
BLOC = B // NCORES
P = 128
NB = BLOC // P
G = N // K  # 128 classes
NAUG = 5
LOG_2PI = math.log(2.0 * math.pi)

F8 = ml_dtypes.float8_e4m3

_CACHE: dict = {}

DEFAULT_CFG = dict(
    n8=2,            # number of 256-row fp8 DoubleRow chunks (0..4)
    dr_layout="A",   # rhs pair layout: "A"=[p,s,n], "B"=[p,n,s]
    wu=12,           # PE warm-up matmuls
    tail_stagger=True,
    stag_tiles=1,
    bufs_work=4,
    bufs_small=6,
    psum_bufs=4,
    out_dtype="float16",
    sem_cap=None,
    group_dep=True,
)


def _build_nc(cfg=None):
    import concourse.bacc as bacc
    import concourse.bass as bass_mod
    import concourse.mybir as mybir
    import concourse.tile as tile
    from concourse.tile import add_dep_helper

    cfg = {**DEFAULT_CFG, **(cfg or {})}
    sem_cap = cfg.get("sem_cap")
    orig_range = bass_mod.get_kernel_semaphore_range
    if sem_cap:
        lo = orig_range().start
        bass_mod.get_kernel_semaphore_range = lambda: range(lo, sem_cap)
    n8 = cfg["n8"]
    nf = (D - 256 * n8) // P  # fp16 full chunks
    NH = N // 512

    f32 = mybir.dt.float32
    f16 = mybir.dt.float16
    f8 = mybir.dt.float8e4
    odt = getattr(mybir.dt, cfg["out_dtype"])

    nc = bacc.Bacc(None, target_bir_lowering=False)
    # Stationary stripes, prepacked per b-tile (partition-major).
    xt16 = nc.dram_tensor("xt16", [NB, P, nf + 1, P], f16, kind="ExternalInput")
    mt16 = nc.dram_tensor("mt16", [nf, P, N], f16, kind="ExternalInput")
    mtaug = nc.dram_tensor("mtaug", [NAUG, N], f16, kind="ExternalInput")
    if n8:
        xt8 = nc.dram_tensor("xt8", [NB, P, n8, 2, P], f8, kind="ExternalInput")
        m8shape = [n8, P, 2, N] if cfg["dr_layout"] == "A" else [n8, P, N, 2]
        mt8 = nc.dram_tensor("mt8", m8shape, f8, kind="ExternalInput")
    out = nc.dram_tensor("out", [BLOC, C], odt, kind="ExternalOutput")

    with tile.TileContext(nc) as tc:
        with (
            tc.tile_pool(name="resident", bufs=1) as resident,
            tc.tile_pool(name="work", bufs=cfg["bufs_work"]) as work,
            tc.tile_pool(name="small", bufs=cfg["bufs_small"]) as small,
            tc.tile_pool(name="psum", bufs=cfg["psum_bufs"], space="PSUM") as psum_pool,
        ):
            xt16_sb = resident.tile([P, NB, nf + 1, P], f16)
            mt16_sb = resident.tile([P, nf + 1, N], f16)
            if n8:
                xt8_sb = resident.tile([P, NB, n8, 2, P], f8)
                mt8_sb = resident.tile(
                    [P, n8, 2, N] if cfg["dr_layout"] == "A" else [P, n8, N, 2], f8
                )

            # PE warm-up during the input-load head (HAM clock gate needs
            # ~3.4us of sustained PE activity to reach 8/8).
            wu_src = resident.tile([P, 512], f16, tag="wu_src")
            nc.gpsimd.memset(wu_src, 0.0)
            wu_ps = psum_pool.tile([P, 512], f32, tag="ps", name="wups")
            for _ in range(cfg["wu"]):
                nc.tensor.matmul(wu_ps, wu_src[:, 0:P], wu_src, start=True, stop=True)

            # Input DMAs, in consumption order.
            nc.sync.dma_start(mt16_sb[0:NAUG, nf, :], mtaug[0:NAUG])
            nc.sync.dma_start(xt16_sb[:, 0], xt16[0])
            if n8:
                nc.sync.dma_start(xt8_sb[:, 0], xt8[0])
            for ch in range(nf):
                nc.sync.dma_start(mt16_sb[:, ch, :], mt16[ch])
            for j in range(n8):
                nc.sync.dma_start(mt8_sb[:, j], mt8[j])
            for bt in range(1, NB):
                nc.sync.dma_start(xt16_sb[:, bt], xt16[bt])
                if n8:
                    nc.sync.dma_start(xt8_sb[:, bt], xt8[bt])

            def mm_ops(bt):
                """(lhsT, rhs_by_half, perf_mode) in stream order."""
                ops = []
                dr = getattr(mybir.MatmulPerfMode, "DoubleRow")
                # aug first (its data arrives earliest)
                ops.append(
                    (
                        xt16_sb[0:NAUG, bt, nf, :],
                        lambda h: mt16_sb[0:NAUG, nf, h * 512 : (h + 1) * 512],
                        None,
                    )
                )
                for ch in range(nf):
                    ops.append(
                        (
                            xt16_sb[:, bt, ch, :],
                            lambda h, ch=ch: mt16_sb[:, ch, h * 512 : (h + 1) * 512],
                            None,
                        )
                    )
                for j in range(n8):
                    if cfg["dr_layout"] == "A":
                        rhs = lambda h, j=j: mt8_sb[:, j, :, h * 512 : (h + 1) * 512]
                    else:
                        rhs = lambda h, j=j: mt8_sb[:, j, h * 512 : (h + 1) * 512, :]
                    ops.append((xt8_sb[:, bt, j], rhs, dr))
                return ops

            pending = []

            def emit_rowlse(bsl, gmaxt, gsumt):
                # out = (ln(gsum) + gmax) - (ln(s2) + rmax),
                # s2 = sum_c gsum_c * exp(gmax_c - rmax)  [u-trick: no lse1
                # dependency, so every ACT op's input is ready in FIFO order]
                nrmax = small.tile([P, 1], f32, tag="nrmax")
                nc.vector.tensor_reduce(
                    nrmax,
                    gmaxt,
                    axis=mybir.AxisListType.X,
                    op=mybir.AluOpType.max,
                    negate=True,
                )
                u = work.tile([P, G], f32, tag="u")
                nc.scalar.activation(
                    u, gmaxt, mybir.ActivationFunctionType.Exp, bias=nrmax
                )
                lseh = work.tile([P, G], f32, tag="lseh")
                nc.scalar.activation(
                    lseh, gsumt, mybir.ActivationFunctionType.Ln
                )
                e2 = work.tile([P, G], f32, tag="e2")
                s2 = small.tile([P, 1], f32, tag="s2")
                nc.vector.scalar_tensor_tensor(
                    e2,
                    gsumt,
                    1.0,
                    u,
                    mybir.AluOpType.mult,
                    mybir.AluOpType.mult,
                    accum_out=s2,
                )
                lse1 = work.tile([P, G], f32, tag="lse1")
                nc.gpsimd.tensor_tensor(lse1, lseh, gmaxt, mybir.AluOpType.add)
                lnz = small.tile([P, 1], f32, tag="lnz")
                nc.scalar.activation(lnz, s2, mybir.ActivationFunctionType.Ln)
                denom_neg = small.tile([P, 1], f32, tag="denom_neg")
                nc.scalar.activation(
                    denom_neg,
                    lnz,
                    mybir.ActivationFunctionType.Identity,
                    bias=nrmax,
                    scale=-1.0,
                )
                ot = work.tile([P, C], odt, tag="ot")
                nc.scalar.activation(
                    ot,
                    lse1,
                    mybir.ActivationFunctionType.Identity,
                    bias=denom_neg,
                )
                nc.sync.dma_start(out[bsl, :], ot)

            last_mm = None
            for bt in range(NB):
                # row-LSE of tile bt-2: all its inputs are >=1 period old, so
                # these ops never stall their engine FIFOs mid-stream
                if len(pending) > 1:
                    emit_rowlse(*pending.pop(0))
                bsl = slice(bt * P, (bt + 1) * P)
                stag = cfg["tail_stagger"] and bt >= NB - cfg["stag_tiles"]
                ops = mm_ops(bt)
                nops = len(ops)

                if stag:
                    pss = [
                        psum_pool.tile([P, 512], f32, tag="ps", name=f"ps{h}")
                        for h in range(NH)
                    ]
                else:
                    ps = psum_pool.tile([P, N], f32, tag="ps")
                    pss = [ps[:, h * 512 : (h + 1) * 512] for h in range(NH)]

                if stag:
                    for h in range(NH):
                        for oi, (lhsT, rhsf, pm) in enumerate(ops):
                            mmi = nc.tensor.matmul(
                                pss[h],
                                lhsT,
                                rhsf(h),
                                start=(oi == 0),
                                stop=(oi == nops - 1),
                                perf_mode=pm,
                            )
                            if oi == 0 and last_mm is not None and cfg["group_dep"]:
                                add_dep_helper(
                                    mmi.ins, last_mm.ins, sync=False,
                                    reason="PE group order",
                                )
                            last_mm = mmi
                else:
                    for oi, (lhsT, rhsf, pm) in enumerate(ops):
                        for h in range(NH):
                            mmi = nc.tensor.matmul(
                                pss[h],
                                lhsT,
                                rhsf(h),
                                start=(oi == 0),
                                stop=(oi == nops - 1),
                                perf_mode=pm,
                            )
                            if oi == 0 and h == 0 and last_mm is not None and cfg["group_dep"]:
                                add_dep_helper(
                                    mmi.ins, last_mm.ins, sync=False,
                                    reason="PE group order",
                                )
                            last_mm = mmi

                # ---- epilogue front: gmax, shift, exp, grouped sum ----
                npieces = NH if stag else 1
                W = N // npieces
                gw = W // K
                gmaxt = work.tile([P, G], f32, tag="gmaxt")
                gsumt = work.tile([P, G], f32, tag="gsumt")
                for hpc in range(npieces):
                    gsl = slice(hpc * gw, (hpc + 1) * gw)
                    if stag:
                        pv = pss[hpc].rearrange("p (g k) -> p g k", k=K)
                    else:
                        pv = ps.rearrange("p (g k) -> p g k", k=K)
                    gmax = gmaxt[:, gsl]
                    nc.vector.tensor_reduce(
                        gmax, pv, axis=mybir.AxisListType.X, op=mybir.AluOpType.max
                    )
                    ei = work.tile([P, W], f32, tag=f"ei{hpc}")
                    nc.vector.tensor_tensor(
                        ei.rearrange("p (g k) -> p g k", k=K),
                        pv,
                        gmax[:, :, None].to_broadcast((P, gw, K)),
                        mybir.AluOpType.subtract,
                    )
                    nc.scalar.activation(ei, ei, mybir.ActivationFunctionType.Exp)
                    eiv = ei.rearrange("p (g k) -> p g k", k=K)
                    t1 = small.tile([P, gw, K // 2], f32, tag=f"t1{hpc}")
                    nc.gpsimd.tensor_tensor(
                        t1, eiv[:, :, 0::2], eiv[:, :, 1::2], mybir.AluOpType.add
                    )
                    t2 = small.tile([P, gw, K // 4], f32, tag=f"t2{hpc}")
                    nc.gpsimd.tensor_tensor(
                        t2, t1[:, :, 0::2], t1[:, :, 1::2], mybir.AluOpType.add
                    )
                    nc.gpsimd.tensor_tensor(
                        gsumt[:, gsl], t2[:, :, 0], t2[:, :, 1], mybir.AluOpType.add
                    )
                pending.append((bsl, gmaxt, gsumt))
            while pending:
                emit_rowlse(*pending.pop(0))

    # Combined Exp+Ln activation table set (avoid per-tile table swaps).
    orig_tables = bacc.get_activation_tables

    def _exp_ln_combined(arch):
        t = orig_tables(arch)
        combined = "natural_log_exp_and_others"
        if combined not in t:
            return t
        strip = {
            mybir.ActivationFunctionType.Exp,
            mybir.ActivationFunctionType.Ln,
        }
        return {
            k: (v if k == combined else (set(v) - strip)) for k, v in t.items()
        }

    bacc.get_activation_tables = _exp_ln_combined
    try:
        nc.compile()
    finally:
        bacc.get_activation_tables = orig_tables
        bass_mod.get_kernel_semaphore_range = orig_range
    return nc


def _split16(v):
    hi = v.astype(np.float16).astype(np.float64)
    lo = v - hi
    return hi, lo


def _host_prep(x, means, bandwidths, weights, priors, cfg=None):
    cfg = {**DEFAULT_CFG, **(cfg or {})}
    n8 = cfg["n8"]
    nf = (D - 256 * n8) // P
    d16 = nf * P

    x = np.asarray(x, dtype=np.float32)
    means = np.asarray(means, dtype=np.float32)

    bw = np.clip(np.asarray(bandwidths, dtype=np.float64), 0.001, 100.0)
    a = 1.0 / bw
    m_sq = np.einsum(
        "nd,nd->n", means.astype(np.float64), means.astype(np.float64)
    )
    w = np.asarray(weights, dtype=np.float64).reshape(C, K)
    log_w = (
        w
        - np.log(np.exp(w - w.max(1, keepdims=True)).sum(1, keepdims=True))
        - w.max(1, keepdims=True)
    ).reshape(N)
    pr = np.asarray(priors, dtype=np.float64)
    log_pri = pr - (np.log(np.exp(pr - pr.max()).sum()) + pr.max())
    cvec = (
        -0.5 * (D * LOG_2PI + D * np.log(bw) + m_sq * a)
        + log_w
        + np.repeat(log_pri, K)
    )
    ah = -0.5 * a
    # Center rank-2 terms (per-row constants cancel in the final LSE).
    ah = ah - ah.mean()
    cvec = cvec - cvec.mean()

    xsq = np.einsum("bd,bd->b", x.astype(np.float64), x.astype(np.float64))
    xsq_h, xsq_l = _split16(xsq)
    ah_h, ah_l = _split16(ah)
    c_h, c_l = _split16(cvec)
    ones = np.ones_like(xsq)

    maT = means.T.astype(np.float64) * a[None, :]  # [D, N]

    # fp16 stationary stripes: [core, bt, p, chunk, col]
    xt16 = np.zeros((NCORES, NB, P, nf + 1, P), dtype=np.float16)
    if nf:
        main = x.T[0:d16].reshape(nf, P, NCORES, NB, P)
        xt16[:, :, :, 0:nf, :] = main.transpose(2, 3, 1, 0, 4).astype(np.float16)
    aug = np.stack([xsq_h, xsq_h, xsq_l, ones, ones]).reshape(NAUG, NCORES, NB, P)
    xt16[:, :, 0:NAUG, nf, :] = aug.transpose(1, 2, 0, 3).astype(np.float16)

    mt16 = maT[0:d16].reshape(nf, P, N).astype(np.float16) if nf else np.zeros(
        (0, P, N), dtype=np.float16
    )
    mtaug = np.stack([ah_h, ah_l, ah_h, c_h, c_l]).astype(np.float16)

    arrs = dict(xt16=xt16, mt16=mt16, mtaug=mtaug)

    if n8:
        # fp8 rows: d in [d16, D), chunk j covers d16 + j*256 + s*128 + p
        x8 = np.clip(x[:, d16:D], -240, 240).astype(F8)
        # [B, 512] -> [core, bt, m, j, s, p] -> [core, bt, p, j, s, m]
        x8p = x8.reshape(NCORES, NB, P, n8, 2, P).transpose(0, 1, 5, 3, 4, 2)
        ma8 = np.clip(maT[d16:D], -240, 240).astype(F8)
        # [512, N] -> [j, s, p, N] -> [j, p, s, n]
        ma8p = ma8.reshape(n8, 2, P, N).transpose(0, 2, 1, 3)
        if cfg["dr_layout"] == "B":
            ma8p = ma8p.transpose(0, 1, 3, 2)
        arrs["xt8"] = np.ascontiguousarray(x8p)
        arrs["mt8"] = np.ascontiguousarray(ma8p)
    return arrs


def _run(x, means, bandwidths, weights, priors, trace=False, cfg=None):
    from concourse.bass_utils import run_bass_kernel_spmd

    key = tuple(sorted({**DEFAULT_CFG, **(cfg or {})}.items()))
    if key not in _CACHE:
        _CACHE[key] = _build_nc(cfg)
    nc = _CACHE[key]

    arrs = _host_prep(x, means, bandwidths, weights, priors, cfg)
    per_core = {k: v for k, v in arrs.items() if k in ("xt16", "xt8")}
    shared = {k: v for k, v in arrs.items() if k not in per_core}
    in_maps = [
        {**{k: np.ascontiguousarray(v[i]) for k, v in per_core.items()}, **shared}
        for i in range(NCORES)
    ]
    res = run_bass_kernel_spmd(
        nc, in_maps, core_ids=list(range(NCORES)), trace=trace
    )
    out = np.concatenate(
        [r["out"].astype(np.float32) for r in res.results], axis=0
    )
    return out, res


def kernel(x, means, bandwidths, weights, priors):
    out, _ = _run(x, means, bandwidths, weights, priors, trace=False)
    return out


# revision 4
# speedup vs baseline: 1.1177x; 1.1177x over previous
"""Trainium2 Bass kernel for spherical deep GMM classifier (DGMMC).

Reference (B=8192, D=1024, C=128 classes, K=8 comps, N=C*K=1024):
    bw = clip(bandwidths, 1e-3, 100); a = 1/bw
    log_prob[b,n] = -0.5*(D*log(2pi) + D*log(bw[n]) + sq_dist[b,n]/bw[n])
    log_prob += log_softmax(weights.reshape(C,K),1).reshape(N)
    lse1[b,c]  = LSE_k(log_prob[b,c*K+k]) + log_softmax(priors)[c]
    out[b,c]   = lse1[b,c] - LSE_c(lse1[b,c])

Data-parallel over batch across 8 cores; per-component affine terms folded
into an augmented GEMM assembled on the host (see _host_prep).

Performance structure:
  - Hybrid-precision GEMM: n8 chunks of 256 contraction rows run as
    fp8e4m3 DoubleRow matmuls (2 contraction rows/PE cell -> half the
    passes of fp16), remaining rows + the rank-2 augmentation stay fp16.
    Hybrid quantization error ~ 2.07e-2 * sqrt(fp8 fraction) ~ 1.47e-2.
  - 12 warm-up matmuls bridge the DMA head so the PE HAM clock-gate
    reaches 8/8 before the real stream (DMA-paced early matmuls alone
    leave it stuck at 4/8).
  - Epilogue split into a per-tile "front" (grouped max / shift / exp /
    gpsimd pairwise-sum tree) and a "row-LSE" emitted two tiles later so
    every op's inputs are ready when it reaches its engine's FIFO head.
  - Row-LSE uses s2 = sum_c gsum_c * exp(gmax_c - rmax) (one DVE
    scalar_tensor_tensor with accumulate) so no op chain runs through
    lse1 before the final affine.
  - Last b-tile runs as two staggered 512-wide half groups so its first
    half's epilogue overlaps the second half's matmuls.
  - fp16 output DMA (upcast on host; quantization adds <1e-3 rel err).
"""

import math

import numpy as np
import ml_dtypes

B, D, C, K = 8192, 1024, 128, 8
N = C * K
NCORES = 8
BLOC = B // NCORES
P = 128
NB = BLOC // P
G = N // K  # 128 classes
NAUG = 5
LOG_2PI = math.log(2.0 * math.pi)

F8 = ml_dtypes.float8_e4m3

_CACHE: dict = {}

DEFAULT_CFG = dict(
    n8=2,            # number of 256-row fp8 DoubleRow chunks (0..4)
    dr_layout="A",   # rhs pair layout: "A"=[p,s,n], "B"=[p,n,s]
    wu=12,           # PE warm-up matmuls
    tail_stagger=True,
    stag_tiles=1,
    bufs_work=4,
    bufs_small=6,
    psum_bufs=4,
    out_dtype="float16",
    sem_cap=None,
    group_dep=True,
)


def _build_nc(cfg=None):
    import concourse.bacc as bacc
    import concourse.bass as bass_mod
    import concourse.mybir as mybir
    import concourse.tile as tile
    from concourse.tile import add_dep_helper

    cfg = {**DEFAULT_CFG, **(cfg or {})}
    sem_cap = cfg.get("sem_cap")
    orig_range = bass_mod.get_kernel_semaphore_range
    if sem_cap:
        lo = orig_range().start
        bass_mod.get_kernel_semaphore_range = lambda: range(lo, sem_cap)
    n8 = cfg["n8"]
    nf = (D - 256 * n8) // P  # fp16 full chunks
    NH = N // 512

    f32 = mybir.dt.float32
    f16 = mybir.dt.float16
    f8 = mybir.dt.float8e4
    odt = getattr(mybir.dt, cfg["out_dtype"])

    nc = bacc.Bacc(None, target_bir_lowering=False)
    # Stationary stripes, prepacked per b-tile (partition-major).
    xt16 = nc.dram_tensor("xt16", [NB, P, nf + 1, P], f16, kind="ExternalInput")
    mt16 = nc.dram_tensor("mt16", [nf, P, N], f16, kind="ExternalInput")
    mtaug = nc.dram_tensor("mtaug", [NAUG, N], f16, kind="ExternalInput")
    if n8:
        xt8 = nc.dram_tensor("xt8", [NB, P, n8, 2, P], f8, kind="ExternalInput")
        m8shape = [n8, P, 2, N] if cfg["dr_layout"] == "A" else [n8, P, N, 2]
        mt8 = nc.dram_tensor("mt8", m8shape, f8, kind="ExternalInput")
    out = nc.dram_tensor("out", [BLOC, C], odt, kind="ExternalOutput")

    with tile.TileContext(nc) as tc:
        with (
            tc.tile_pool(name="resident", bufs=1) as resident,
            tc.tile_pool(name="work", bufs=cfg["bufs_work"]) as work,
            tc.tile_pool(name="small", bufs=cfg["bufs_small"]) as small,
            tc.tile_pool(name="psum", bufs=cfg["psum_bufs"], space="PSUM") as psum_pool,
        ):
            xt16_sb = resident.tile([P, NB, nf + 1, P], f16)
            mt16_sb = resident.tile([P, nf + 1, N], f16)
            if n8:
                xt8_sb = resident.tile([P, NB, n8, 2, P], f8)
                mt8_sb = resident.tile(
                    [P, n8, 2, N] if cfg["dr_layout"] == "A" else [P, n8, N, 2], f8
                )

            # PE warm-up during the input-load head (HAM clock gate needs
            # ~3.4us of sustained PE activity to reach 8/8).
            wu_src = resident.tile([P, 512], f16, tag="wu_src")
            nc.gpsimd.memset(wu_src, 0.0)
            wu_ps = psum_pool.tile([P, 512], f32, tag="ps", name="wups")
            for _ in range(cfg["wu"]):
                nc.tensor.matmul(wu_ps, wu_src[:, 0:P], wu_src, start=True, stop=True)

            # Input DMAs, in consumption order.
            nc.sync.dma_start(mt16_sb[0:NAUG, nf, :], mtaug[0:NAUG])
            nc.sync.dma_start(xt16_sb[:, 0], xt16[0])
            if n8:
                nc.sync.dma_start(xt8_sb[:, 0], xt8[0])
            for ch in range(nf):
                nc.sync.dma_start(mt16_sb[:, ch, :], mt16[ch])
            for j in range(n8):
                nc.sync.dma_start(mt8_sb[:, j], mt8[j])
            for bt in range(1, NB):
                nc.sync.dma_start(xt16_sb[:, bt], xt16[bt])
                if n8:
                    nc.sync.dma_start(xt8_sb[:, bt], xt8[bt])

            def mm_ops(bt):
                """(lhsT, rhs_by_half, perf_mode) in stream order."""
                ops = []
                dr = getattr(mybir.MatmulPerfMode, "DoubleRow")
                # aug first (its data arrives earliest)
                ops.append(
                    (
                        xt16_sb[0:NAUG, bt, nf, :],
                        lambda h: mt16_sb[0:NAUG, nf, h * 512 : (h + 1) * 512],
                        None,
                    )
                )
                for ch in range(nf):
                    ops.append(
                        (
                            xt16_sb[:, bt, ch, :],
                            lambda h, ch=ch: mt16_sb[:, ch, h * 512 : (h + 1) * 512],
                            None,
                        )
                    )
                for j in range(n8):
                    if cfg["dr_layout"] == "A":
                        rhs = lambda h, j=j: mt8_sb[:, j, :, h * 512 : (h + 1) * 512]
                    else:
                        rhs = lambda h, j=j: mt8_sb[:, j, h * 512 : (h + 1) * 512, :]
                    ops.append((xt8_sb[:, bt, j], rhs, dr))
                return ops

            pending = []

            def emit_rowlse(bsl, gmaxt, gsumt, gsum_work=()):
                for gsl_l, eiv_l in gsum_work:
                    nc.vector.tensor_reduce(
                        gsumt[:, gsl_l],
                        eiv_l,
                        axis=mybir.AxisListType.X,
                        op=mybir.AluOpType.add,
                    )
                # out = (ln(gsum) + gmax) - (ln(s2) + rmax),
                # s2 = sum_c gsum_c * exp(gmax_c - rmax)  [u-trick: no lse1
                # dependency, so every ACT op's input is ready in FIFO order]
                nrmax = small.tile([P, 1], f32, tag="nrmax")
                nc.vector.tensor_reduce(
                    nrmax,
                    gmaxt,
                    axis=mybir.AxisListType.X,
                    op=mybir.AluOpType.max,
                    negate=True,
                )
                u = work.tile([P, G], f32, tag="u")
                nc.scalar.activation(
                    u, gmaxt, mybir.ActivationFunctionType.Exp, bias=nrmax
                )
                lseh = work.tile([P, G], f32, tag="lseh")
                nc.scalar.activation(
                    lseh, gsumt, mybir.ActivationFunctionType.Ln
                )
                e2 = work.tile([P, G], f32, tag="e2")
                s2 = small.tile([P, 1], f32, tag="s2")
                nc.vector.scalar_tensor_tensor(
                    e2,
                    gsumt,
                    1.0,
                    u,
                    mybir.AluOpType.mult,
                    mybir.AluOpType.mult,
                    accum_out=s2,
                )
                lse1 = work.tile([P, G], f32, tag="lse1")
                nc.gpsimd.tensor_tensor(lse1, lseh, gmaxt, mybir.AluOpType.add)
                lnz = small.tile([P, 1], f32, tag="lnz")
                nc.scalar.activation(lnz, s2, mybir.ActivationFunctionType.Ln)
                denom_neg = small.tile([P, 1], f32, tag="denom_neg")
                nc.scalar.activation(
                    denom_neg,
                    lnz,
                    mybir.ActivationFunctionType.Identity,
                    bias=nrmax,
                    scale=-1.0,
                )
                ot = work.tile([P, C], odt, tag="ot")
                nc.scalar.activation(
                    ot,
                    lse1,
                    mybir.ActivationFunctionType.Identity,
                    bias=denom_neg,
                )
                nc.sync.dma_start(out[bsl, :], ot)

            last_mm = None
            for bt in range(NB):
                # row-LSE of tile bt-2: all its inputs are >=1 period old, so
                # these ops never stall their engine FIFOs mid-stream
                if len(pending) > 1:
                    emit_rowlse(*pending.pop(0))
                bsl = slice(bt * P, (bt + 1) * P)
                stag = cfg["tail_stagger"] and bt >= NB - cfg["stag_tiles"]
                ops = mm_ops(bt)
                nops = len(ops)

                if stag:
                    pss = [
                        psum_pool.tile([P, 512], f32, tag="ps", name=f"ps{h}")
                        for h in range(NH)
                    ]
                else:
                    ps = psum_pool.tile([P, N], f32, tag="ps")
                    pss = [ps[:, h * 512 : (h + 1) * 512] for h in range(NH)]

                if stag:
                    for h in range(NH):
                        for oi, (lhsT, rhsf, pm) in enumerate(ops):
                            mmi = nc.tensor.matmul(
                                pss[h],
                                lhsT,
                                rhsf(h),
                                start=(oi == 0),
                                stop=(oi == nops - 1),
                                perf_mode=pm,
                            )
                            if oi == 0 and last_mm is not None and cfg["group_dep"]:
                                add_dep_helper(
                                    mmi.ins, last_mm.ins, sync=False,
                                    reason="PE group order",
                                )
                            last_mm = mmi
                else:
                    for oi, (lhsT, rhsf, pm) in enumerate(ops):
                        for h in range(NH):
                            mmi = nc.tensor.matmul(
                                pss[h],
                                lhsT,
                                rhsf(h),
                                start=(oi == 0),
                                stop=(oi == nops - 1),
                                perf_mode=pm,
                            )
                            if oi == 0 and h == 0 and last_mm is not None and cfg["group_dep"]:
                                add_dep_helper(
                                    mmi.ins, last_mm.ins, sync=False,
                                    reason="PE group order",
                                )
                            last_mm = mmi

                # ---- epilogue front: gmax, shift, exp, grouped sum ----
                npieces = NH if stag else 1
                W = N // npieces
                gw = W // K
                late = bt == NB - 1
                half_late = bt >= NB - 3 and not late
                gsum_work = []
                gmaxt = work.tile([P, G], f32, tag="gmaxt")
                gsumt = work.tile([P, G], f32, tag="gsumt")
                for hpc in range(npieces):
                    gsl = slice(hpc * gw, (hpc + 1) * gw)
                    if stag:
                        pv = pss[hpc].rearrange("p (g k) -> p g k", k=K)
                    else:
                        pv = ps.rearrange("p (g k) -> p g k", k=K)
                    gmax = gmaxt[:, gsl]
                    nc.vector.tensor_reduce(
                        gmax, pv, axis=mybir.AxisListType.X, op=mybir.AluOpType.max
                    )
                    ei = work.tile([P, W], f32, tag=f"ei{hpc}")
                    nc.vector.tensor_tensor(
                        ei.rearrange("p (g k) -> p g k", k=K),
                        pv,
                        gmax[:, :, None].to_broadcast((P, gw, K)),
                        mybir.AluOpType.subtract,
                    )
                    nc.scalar.activation(ei, ei, mybir.ActivationFunctionType.Exp)
                    eiv = ei.rearrange("p (g k) -> p g k", k=K)
                    if late:
                        gsum_work.append((gsl, eiv))
                        continue
                    t1 = small.tile([P, gw, K // 2], f32, tag=f"t1{hpc}")
                    nc.gpsimd.tensor_tensor(
                        t1, eiv[:, :, 0::2], eiv[:, :, 1::2], mybir.AluOpType.add
                    )
                    if half_late:
                        # finish this tile's grouped sum in the drain (DVE
                        # is idle there); t1 stays on GPSIMD so ei's
                        # lifetime is unchanged
                        gsum_work.append((gsl, t1))
                        continue
                    t2 = small.tile([P, gw, K // 4], f32, tag=f"t2{hpc}")
                    nc.gpsimd.tensor_tensor(
                        t2, t1[:, :, 0::2], t1[:, :, 1::2], mybir.AluOpType.add
                    )
                    nc.gpsimd.tensor_tensor(
                        gsumt[:, gsl], t2[:, :, 0], t2[:, :, 1], mybir.AluOpType.add
                    )
                pending.append((bsl, gmaxt, gsumt, gsum_work))
            while pending:
                emit_rowlse(*pending.pop(0))

    # Combined Exp+Ln activation table set (avoid per-tile table swaps).
    orig_tables = bacc.get_activation_tables

    def _exp_ln_combined(arch):
        t = orig_tables(arch)
        combined = "natural_log_exp_and_others"
        if combined not in t:
            return t
        strip = {
            mybir.ActivationFunctionType.Exp,
            mybir.ActivationFunctionType.Ln,
        }
        return {
            k: (v if k == combined else (set(v) - strip)) for k, v in t.items()
        }

    bacc.get_activation_tables = _exp_ln_combined
    try:
        nc.compile()
    finally:
        bacc.get_activation_tables = orig_tables
        bass_mod.get_kernel_semaphore_range = orig_range
    return nc


def _split16(v):
    hi = v.astype(np.float16).astype(np.float64)
    lo = v - hi
    return hi, lo


def _host_prep(x, means, bandwidths, weights, priors, cfg=None):
    cfg = {**DEFAULT_CFG, **(cfg or {})}
    n8 = cfg["n8"]
    nf = (D - 256 * n8) // P
    d16 = nf * P

    x = np.asarray(x, dtype=np.float32)
    means = np.asarray(means, dtype=np.float32)

    bw = np.clip(np.asarray(bandwidths, dtype=np.float64), 0.001, 100.0)
    a = 1.0 / bw
    m_sq = np.einsum(
        "nd,nd->n", means.astype(np.float64), means.astype(np.float64)
    )
    w = np.asarray(weights, dtype=np.float64).reshape(C, K)
    log_w = (
        w
        - np.log(np.exp(w - w.max(1, keepdims=True)).sum(1, keepdims=True))
        - w.max(1, keepdims=True)
    ).reshape(N)
    pr = np.asarray(priors, dtype=np.float64)
    log_pri = pr - (np.log(np.exp(pr - pr.max()).sum()) + pr.max())
    cvec = (
        -0.5 * (D * LOG_2PI + D * np.log(bw) + m_sq * a)
        + log_w
        + np.repeat(log_pri, K)
    )
    ah = -0.5 * a
    # Center rank-2 terms (per-row constants cancel in the final LSE).
    ah = ah - ah.mean()
    cvec = cvec - cvec.mean()

    xsq = np.einsum("bd,bd->b", x.astype(np.float64), x.astype(np.float64))
    xsq_h, xsq_l = _split16(xsq)
    ah_h, ah_l = _split16(ah)
    c_h, c_l = _split16(cvec)
    ones = np.ones_like(xsq)

    maT = means.T.astype(np.float64) * a[None, :]  # [D, N]

    # fp16 stationary stripes: [core, bt, p, chunk, col]
    xt16 = np.zeros((NCORES, NB, P, nf + 1, P), dtype=np.float16)
    if nf:
        main = x.T[0:d16].reshape(nf, P, NCORES, NB, P)
        xt16[:, :, :, 0:nf, :] = main.transpose(2, 3, 1, 0, 4).astype(np.float16)
    aug = np.stack([xsq_h, xsq_h, xsq_l, ones, ones]).reshape(NAUG, NCORES, NB, P)
    xt16[:, :, 0:NAUG, nf, :] = aug.transpose(1, 2, 0, 3).astype(np.float16)

    mt16 = maT[0:d16].reshape(nf, P, N).astype(np.float16) if nf else np.zeros(
        (0, P, N), dtype=np.float16
    )
    mtaug = np.stack([ah_h, ah_l, ah_h, c_h, c_l]).astype(np.float16)

    arrs = dict(xt16=xt16, mt16=mt16, mtaug=mtaug)

    if n8:
        # fp8 rows: d in [d16, D), chunk j covers d16 + j*256 + s*128 + p
        x8 = np.clip(x[:, d16:D], -240, 240).astype(F8)
        # [B, 512] -> [core, bt, m, j, s, p] -> [core, bt, p, j, s, m]
        x8p = x8.reshape(NCORES, NB, P, n8, 2, P).transpose(0, 1, 5, 3, 4, 2)
        ma8 = np.clip(maT[d16:D], -240, 240).astype(F8)
        # [512, N] -> [j, s, p, N] -> [j, p, s, n]
        ma8p = ma8.reshape(n8, 2, P, N).transpose(0, 2, 1, 3)
        if cfg["dr_layout"] == "B":
            ma8p = ma8p.transpose(0, 1, 3, 2)
        arrs["xt8"] = np.ascontiguousarray(x8p)
        arrs["mt8"] = np.ascontiguousarray(ma8p)
    return arrs


def _run(x, means, bandwidths, weights, priors, trace=False, cfg=None):
    from concourse.bass_utils import run_bass_kernel_spmd

    key = tuple(sorted({**DEFAULT_CFG, **(cfg or {})}.items()))
    if key not in _CACHE:
        _CACHE[key] = _build_nc(cfg)
    nc = _CACHE[key]

    arrs = _host_prep(x, means, bandwidths, weights, priors, cfg)
    per_core = {k: v for k, v in arrs.items() if k in ("xt16", "xt8")}
    shared = {k: v for k, v in arrs.items() if k not in per_core}
    in_maps = [
        {**{k: np.ascontiguousarray(v[i]) for k, v in per_core.items()}, **shared}
        for i in range(NCORES)
    ]
    res = run_bass_kernel_spmd(
        nc, in_maps, core_ids=list(range(NCORES)), trace=trace
    )
    out = np.concatenate(
        [r["out"].astype(np.float32) for r in res.results], axis=0
    )
    return out, res


def kernel(x, means, bandwidths, weights, priors):
    out, _ = _run(x, means, bandwidths, weights, priors, trace=False)
    return out
